# revision 1
# baseline (speedup 1.0000x reference)
"""Trainium2 Bass kernel for a 2-layer GCN (EnhancedHockeyGNN).

Strategy (8 NeuronCores, SPMD, two NEFF launches):
  - Nodes sharded row-wise across cores (dst ownership); weights replicated.
  - Per layer: xs = (x @ W) * dinv computed on the owning core in fp32 and
    staged to DRAM as an fp16 message table, AllGather'd (in 4 overlapped
    chunks) into every core's DRAM.
  - Edges (incl. explicit self-loops) are sharded by dst owner and bin-packed
    into groups of <=128 dst nodes / <=2048 edges. Per 128-edge tile the src
    rows are fetched with an indirect DMA (one row per partition); a one-hot
    matrix (value dinv[dst], built wide on the vector engine) turns the
    segment-sum into a PSUM-accumulated fp16 matmul chain producing
    feature-major aggregates.
  - BN+ReLU (eval) folds into one scalar-engine activation per group.
  - Layer-2's message table is produced and AllGather'd inside part A (hidden
    under the layer-1 gather stream) and handed to part B as a plain input,
    so part B starts gathering immediately.
  - Readout computes log-softmax for every node; the host selects the
    requested game_indices rows (pure index routing).
"""
import math

import numpy as np

# ---------------------------------------------------------------- constants
N = 100000
F_IN = 128
H = 128
NC = 8
SHARD = 12544            # multiple of 128; 8 * 12544 = 100352 >= N
NPAD = NC * SHARD
GROUP_EDGES = 2048       # edges per group (16 tiles of 128)
GROUP_TILES = GROUP_EDGES // 128
GROUP_DSTS = 128         # max dst nodes per group (PSUM partition dim)
NCHUNK = 4               # AllGather overlap chunks
EPS = 1e-5

_CACHE = {}


def _chunks(n, k):
    """Split n items into k nearly-equal contiguous chunks: list of (lo, hi)."""
    k = min(k, n)
    base = n // k
    rem = n % k
    out = []
    lo = 0
    for i in range(k):
        hi = lo + base + (1 if i < rem else 0)
        out.append((lo, hi))
        lo = hi
    return out


# ---------------------------------------------------------------- host prep
def _bin_pack(counts, G):
    order = np.argsort(-counts, kind="stable")
    bin_edges = np.zeros(G, dtype=np.int64)
    bin_nodes = np.zeros(G, dtype=np.int64)
    group_of = np.full(counts.shape[0], -1, dtype=np.int32)
    pos_in_group = np.full(counts.shape[0], -1, dtype=np.int32)
    for d in order:
        c = counts[d]
        placed = False
        for b in range(G):
            if bin_edges[b] + c <= GROUP_EDGES and bin_nodes[b] < GROUP_DSTS:
                group_of[d] = b
                pos_in_group[d] = bin_nodes[b]
                bin_edges[b] += c
                bin_nodes[b] += 1
                placed = True
                break
        if not placed:
            return None
    return group_of, pos_in_group


def _prepare(x, edge_index, cfg):
    n, npad, shard, nc = cfg["N"], cfg["NPAD"], cfg["SHARD"], cfg["NC"]
    ge, gt = cfg["GROUP_EDGES"], cfg["GROUP_TILES"]

    src = np.asarray(edge_index[0], dtype=np.int64)
    dst = np.asarray(edge_index[1], dtype=np.int64)
    deg = np.bincount(dst, minlength=n).astype(np.float64) + 1.0
    dinv = (1.0 / np.sqrt(deg)).astype(np.float32)
    dinv_pad_full = np.ones(npad, dtype=np.float32)
    dinv_pad_full[:n] = dinv

    sall = np.concatenate([src, np.arange(n, dtype=np.int64)])
    dall = np.concatenate([dst, np.arange(n, dtype=np.int64)])
    owner = dall // shard

    Es = [int((owner == c).sum()) for c in range(nc)]
    G = max(int(math.ceil(e / ge)) for e in Es)
    while True:
        packs = []
        ok = True
        for c in range(nc):
            m = owner == c
            d0 = (dall[m] - c * shard).astype(np.int64)
            counts = np.bincount(d0, minlength=shard)
            r = _bin_pack(counts, G)
            if r is None:
                ok = False
                break
            packs.append((r[0], r[1], d0, sall[m]))
        if ok:
            break
        G += 1

    ntiles = G * gt
    ntile_nat = shard // 128
    nchunk = cfg["NCHUNK"]

    # ----- chunk-major table layouts (AllGather per chunk)
    ch_a = _chunks(ntile_nat, nchunk)
    chunk_base_a = []
    acc = 0
    for lo, hi in ch_a:
        chunk_base_a.append(acc)
        acc += nc * (hi - lo) * 128
    tile_q = np.zeros(ntile_nat, dtype=np.int64)
    for q, (lo, hi) in enumerate(ch_a):
        tile_q[lo:hi] = q
    nodes = np.arange(npad, dtype=np.int64)
    c_of = nodes // shard
    loc = nodes % shard
    j_of = loc // 128
    p_of = loc % 128
    q_of = tile_q[j_of]
    rows_q = np.array([hi - lo for lo, hi in ch_a], dtype=np.int64)[q_of] * 128
    lo_q = np.array([lo for lo, hi in ch_a], dtype=np.int64)[q_of]
    base_q = np.array(chunk_base_a, dtype=np.int64)[q_of]
    xs1_row = base_q + c_of * rows_q + (j_of - lo_q) * 128 + p_of

    ch_d = _chunks(G, nchunk)
    chunk_base_d = []
    acc = 0
    for lo, hi in ch_d:
        chunk_base_d.append(acc)
        acc += nc * (hi - lo) * 128
    g_q = np.zeros(G, dtype=np.int64)
    for q, (lo, hi) in enumerate(ch_d):
        g_q[lo:hi] = q

    padded_row = np.zeros(npad, dtype=np.int64)      # node -> xs2 row
    pad_cji = np.zeros((npad, 3), dtype=np.int64)    # node -> (core, group, pos)
    for c in range(nc):
        group_of, pos, _, _ = packs[c]
        g64 = group_of.astype(np.int64)
        p64 = pos.astype(np.int64)
        q = g_q[g64]
        lo = np.array([l for l, _ in ch_d], dtype=np.int64)[q]
        hi = np.array([h_ for _, h_ in ch_d], dtype=np.int64)[q]
        base = np.array(chunk_base_d, dtype=np.int64)[q]
        rows = c * (hi - lo) * 128 + (g64 - lo) * 128 + p64 + base
        padded_row[c * shard:(c + 1) * shard] = rows
        pad_cji[c * shard:(c + 1) * shard, 0] = c
        pad_cji[c * shard:(c + 1) * shard, 1] = g64
        pad_cji[c * shard:(c + 1) * shard, 2] = p64

    per_core = []
    for c in range(nc):
        group_of, pos, d0, s_nodes = packs[c]
        g_of_edge = group_of[d0]
        order = np.argsort(g_of_edge, kind="stable")
        d0o, so, go = d0[order], s_nodes[order], g_of_edge[order]
        src1 = np.zeros((128, ntiles), dtype=np.int32)
        src2 = np.zeros((128, ntiles), dtype=np.int32)
        dloc = np.full((128, ntiles), 300.0, dtype=np.float16)
        dinv_dst = np.zeros((128, ntiles), dtype=np.float16)
        gstart = np.searchsorted(go, np.arange(G))
        gend = np.searchsorted(go, np.arange(G) + 1)
        for g in range(G):
            a, b = int(gstart[g]), int(gend[g])
            k = b - a
            assert k <= ge
            sl_s = so[a:b]
            sl_d = d0o[a:b]
            t = np.arange(k) // 128
            p = np.arange(k) % 128
            cols = g * gt + t
            src1[p, cols] = xs1_row[sl_s]
            src2[p, cols] = padded_row[sl_s]
            dloc[p, cols] = pos[sl_d]
            dinv_dst[p, cols] = dinv_pad_full[c * shard + sl_d]
        jj = np.arange(shard)
        dinv_nat = dinv_pad_full[c * shard + jj].reshape(shard // 128, 128).T.copy()
        xs_shape = np.zeros((shard, x.shape[1]), dtype=np.float32)
        lo, hi = c * shard, min((c + 1) * shard, n)
        xs_shape[: hi - lo] = x[lo:hi]
        xT = np.ascontiguousarray(xs_shape.T)
        inv_nodes = np.full(G * 128, -1, dtype=np.int64)
        inv_nodes[group_of.astype(np.int64) * 128 + pos.astype(np.int64)] = \
            np.arange(shard)
        valid = inv_nodes >= 0
        vals = np.zeros(G * 128, dtype=np.float32)
        vals[valid] = dinv_pad_full[c * shard + inv_nodes[valid]]
        dinv_padlay = vals.reshape(G, 128).T.copy()
        per_core.append(dict(src1=src1, src2=src2, dloc=dloc, dinv_dst=dinv_dst,
                             dinv_nat=dinv_nat, dinv_padlay=dinv_padlay, xT=xT))
    meta = dict(ch_a=ch_a, ch_d=ch_d, pad_cji=pad_cji)
    return per_core, meta, G, ntiles


def _fold_bn(gamma, beta, mean, var, b):
    s = (gamma / np.sqrt(var + EPS)).astype(np.float32)
    t = ((b - mean) * s + beta).astype(np.float32)
    return s.reshape(H, 1), t.reshape(H, 1)


# ---------------------------------------------------------------- bass build
def _build(cfg, G, ntiles, part, meta):
    import concourse.bacc as bacc
    import concourse.bass as bass
    import concourse.mybir as mybir
    import concourse.tile as tile

    fp32 = mybir.dt.float32
    fp16 = mybir.dt.float16
    i32 = mybir.dt.int32
    AF = mybir.ActivationFunctionType

    nc_ = cfg["NC"]
    shard = cfg["SHARD"]
    gt = cfg["GROUP_TILES"]
    ntile_nat = shard // 128
    h = cfg["H"]
    fin = cfg["F_IN"]
    ch_a = meta["ch_a"]
    ch_d = meta["ch_d"]
    xs1_rows = nc_ * ntile_nat * 128
    xs2_rows = nc_ * G * 128

    nc = bacc.Bacc(None, target_bir_lowering=False, debug=False, num_devices=nc_)

    iota_in = nc.dram_tensor("iota", [128, 128], fp16, kind="ExternalInput")
    dloc_in = nc.dram_tensor("dloc", [128, ntiles], fp16, kind="ExternalInput")
    dd_in = nc.dram_tensor("dinv_dst", [128, ntiles], fp16, kind="ExternalInput")

    if part == "a":
        xT_in = nc.dram_tensor("xT", [fin, shard], fp32, kind="ExternalInput")
        w1_in = nc.dram_tensor("W1", [fin, h], fp32, kind="ExternalInput")
        w2_in = nc.dram_tensor("W2", [h, h], fp32, kind="ExternalInput")
        s1_in = nc.dram_tensor("s1", [h, 1], fp32, kind="ExternalInput")
        t1_in = nc.dram_tensor("t1", [h, 1], fp32, kind="ExternalInput")
        src1_in = nc.dram_tensor("src1", [128, ntiles], i32, kind="ExternalInput")
        dn_in = nc.dram_tensor("dinv_nat", [128, ntile_nat], fp32,
                               kind="ExternalInput")
        dp_in = nc.dram_tensor("dinv_padlay", [128, G], fp32,
                               kind="ExternalInput")
        out_xs2 = nc.dram_tensor("xs2_full_out", [xs2_rows, h], fp16,
                                 kind="ExternalOutput")
    else:
        xs2_in = nc.dram_tensor("xs2_full_in", [xs2_rows, h], fp16,
                                kind="ExternalInput")
        wf_in = nc.dram_tensor("Wf", [h, 2], fp32, kind="ExternalInput")
        bf_in = nc.dram_tensor("bf_rep", [128, 2], fp32, kind="ExternalInput")
        s2_in = nc.dram_tensor("s2", [h, 1], fp32, kind="ExternalInput")
        t2_in = nc.dram_tensor("t2", [h, 1], fp32, kind="ExternalInput")
        src2_in = nc.dram_tensor("src2", [128, ntiles], i32, kind="ExternalInput")
        out_lp = nc.dram_tensor("logp", [128, 2 * G], fp32, kind="ExternalOutput")

    with tile.TileContext(nc) as tc:
        with (
            tc.tile_pool(name="res", bufs=1) as res,
            tc.tile_pool(name="big", bufs=1) as big,
            tc.tile_pool(name="stream", bufs=1) as st,
            tc.tile_pool(name="ps", bufs=1, space="PSUM") as ps,
            tc.tile_pool(name="dram", bufs=1, space="DRAM") as dram,
        ):
            iota_t = res.tile([128, 128], fp16)
            dloc_t = res.tile([128, ntiles], fp16)
            dd_t = res.tile([128, ntiles], fp16)
            nc.sync.dma_start(out=iota_t[:], in_=iota_in[:])
            nc.sync.dma_start(out=dloc_t[:], in_=dloc_in[:])
            nc.sync.dma_start(out=dd_t[:], in_=dd_in[:])

            def edge_layer(src_t, xs_full_ap, s_t, t_t, hT, post_group=None):
                for g in range(G):
                    oh = st.tile([128, gt, 128], fp16, name="oh", tag="oh",
                                 bufs=3)
                    nc.vector.tensor_tensor(
                        out=oh[:],
                        in0=dloc_t[:, g * gt:(g + 1) * gt].to_broadcast(
                            [128, gt, 128]),
                        in1=bass.AP(iota_t[:].tensor, iota_t[:].offset,
                                    [iota_t[:].ap[0], [0, gt], [1, 128]]),
                        op=mybir.AluOpType.is_equal,
                    )
                    nc.vector.tensor_tensor(
                        out=oh[:],
                        in0=oh[:],
                        in1=dd_t[:, g * gt:(g + 1) * gt].to_broadcast(
                            [128, gt, 128]),
                        op=mybir.AluOpType.mult,
                    )
                    pg = ps.tile([h, 128], fp32, name="pg", tag="pg", bufs=4)
                    for t in range(gt):
                        k = g * gt + t
                        msg = st.tile([128, h], fp16, name="msg", tag="msg",
                                      bufs=16)
                        nc.gpsimd.indirect_dma_start(
                            out=msg[:],
                            out_offset=None,
                            in_=xs_full_ap,
                            in_offset=bass.IndirectOffsetOnAxis(
                                ap=src_t[:, k:k + 1], axis=0),
                        )
                        nc.tensor.matmul(pg[:], msg[:], oh[:, t, :],
                                         start=(t == 0), stop=(t == gt - 1))
                    nc.scalar.activation(
                        out=hT[:, g * 128:(g + 1) * 128], in_=pg[:],
                        func=AF.Relu, bias=t_t[:], scale=s_t[:],
                    )
                    if post_group is not None:
                        post_group(g)

            if part == "a":
                w1_t = res.tile([fin, h], fp32)
                w2_t = res.tile([h, h], fp32)
                s1_t = res.tile([h, 1], fp32)
                t1_t = res.tile([h, 1], fp32)
                src1_t = res.tile([128, ntiles], i32)
                dn_t = res.tile([128, ntile_nat], fp32)
                dp_t = res.tile([128, G], fp32)
                for t_, i_ in ((w1_t, w1_in), (w2_t, w2_in), (s1_t, s1_in),
                               (t1_t, t1_in), (src1_t, src1_in), (dn_t, dn_in),
                               (dp_t, dp_in)):
                    nc.sync.dma_start(out=t_[:], in_=i_[:])

                xs1_shard = dram.tile([shard, h], fp16)
                xs1_full = dram.tile([xs1_rows, h], fp16)
                xs2_shard = dram.tile([G * 128, h], fp16)
                xs2_full = dram.tile([xs2_rows, h], fp16)

                # ---- stage A: xs1 tiles, staged + AllGather'd per chunk
                xsb = big.tile([128, ntile_nat * 128], fp16, name="xsb",
                               tag="big_a")
                row_base = 0
                for q, (lo, hi) in enumerate(ch_a):
                    for j in range(lo, hi):
                        lhsT = st.tile([128, 128], fp32, name="xTt",
                                       tag="lhsT", bufs=4)
                        nc.sync.dma_start(
                            out=lhsT[:], in_=xT_in[:, j * 128:(j + 1) * 128])
                        pxs = ps.tile([128, h], fp32, name="pxs", tag="pxs",
                                      bufs=2)
                        nc.tensor.matmul(pxs[:], lhsT[:], w1_t[:], start=True,
                                         stop=True)
                        nc.vector.tensor_scalar(
                            out=xsb[:, j * 128:(j + 1) * 128], in0=pxs[:],
                            scalar1=dn_t[:, j:j + 1], scalar2=None,
                            op0=mybir.AluOpType.mult)
                    rows = hi - lo
                    dest = bass.AP(xs1_shard[:].tensor, lo * 128 * h,
                                   [[h, 128], [128 * h, rows], [1, h]])
                    nc.sync.dma_start(out=dest, in_=xsb[:].rearrange(
                        "p (j f) -> p j f", f=h)[:, lo:hi, :])
                    nc.gpsimd.collective_compute(
                        "AllGather", mybir.AluOpType.bypass,
                        replica_groups=[list(range(nc_))],
                        ins=[xs1_shard[lo * 128:hi * 128, :].opt()],
                        outs=[xs1_full[row_base:row_base + nc_ * rows * 128,
                                       :].opt()],
                    )
                    row_base += nc_ * rows * 128

                # ---- stage C (layer 1) with interleaved stage D (xs2 + AG2)
                hT = big.tile([128, G * 128], fp32, name="hT", tag="big_b")
                xs2b = big.tile([128, G * 128], fp16, name="xs2b", tag="big_c")
                g_meta = {}
                acc = 0
                for q, (lo, hi) in enumerate(ch_d):
                    for g in range(lo, hi):
                        g_meta[g] = (q, lo, hi, acc)
                    acc += nc_ * (hi - lo) * 128

                def post_group(g):
                    pxs = ps.tile([128, h], fp32, name="pxs2", tag="pxs",
                                  bufs=2)
                    nc.tensor.matmul(pxs[:], hT[:, g * 128:(g + 1) * 128],
                                     w2_t[:], start=True, stop=True)
                    nc.vector.tensor_scalar(
                        out=xs2b[:, g * 128:(g + 1) * 128], in0=pxs[:],
                        scalar1=dp_t[:, g:g + 1], scalar2=None,
                        op0=mybir.AluOpType.mult)
                    q, lo, hi, dbase = g_meta[g]
                    if g == hi - 1:
                        rows = hi - lo
                        dest = bass.AP(xs2_shard[:].tensor, lo * 128 * h,
                                       [[h, 128], [128 * h, rows], [1, h]])
                        nc.sync.dma_start(out=dest, in_=xs2b[:].rearrange(
                            "p (j f) -> p j f", f=h)[:, lo:hi, :])
                        nc.gpsimd.collective_compute(
                            "AllGather", mybir.AluOpType.bypass,
                            replica_groups=[list(range(nc_))],
                            ins=[xs2_shard[lo * 128:hi * 128, :].opt()],
                            outs=[xs2_full[dbase:dbase + nc_ * rows * 128,
                                           :].opt()],
                        )

                edge_layer(src1_t, xs1_full[:], s1_t, t1_t, hT, post_group)
                nc.sync.dma_start(out=out_xs2[:], in_=xs2_full[:])
            else:
                wf_t = res.tile([h, 2], fp32)
                bf_t = res.tile([128, 2], fp32)
                s2_t = res.tile([h, 1], fp32)
                t2_t = res.tile([h, 1], fp32)
                src2_t = res.tile([128, ntiles], i32)
                for t_, i_ in ((wf_t, wf_in), (bf_t, bf_in), (s2_t, s2_in),
                               (t2_t, t2_in), (src2_t, src2_in)):
                    nc.sync.dma_start(out=t_[:], in_=i_[:])

                h2T = big.tile([128, G * 128], fp32, name="h2T", tag="big_b")
                edge_layer(src2_t, xs2_in[:], s2_t, t2_t, h2T)

                lg = res.tile([128, 2 * G], fp32)
                for j in range(G):
                    plg = ps.tile([128, 2], fp32, name="plg", tag="plg", bufs=2)
                    nc.tensor.matmul(plg[:], h2T[:, j * 128:(j + 1) * 128],
                                     wf_t[:], start=True, stop=True)
                    nc.vector.tensor_add(out=lg[:, 2 * j:2 * j + 2],
                                         in0=plg[:], in1=bf_t[:])

                def strided(base, start):
                    a = base[:]
                    return bass.AP(a.tensor, a.offset + start,
                                   [a.ap[0], [2, G]])

                z0, z1 = strided(lg, 0), strided(lg, 1)
                mx = res.tile([128, G], fp32)
                nc.vector.tensor_tensor(out=mx[:], in0=z0, in1=z1,
                                        op=mybir.AluOpType.max)
                sm0 = res.tile([128, G], fp32)
                sm1 = res.tile([128, G], fp32)
                nc.vector.tensor_sub(out=sm0[:], in0=z0, in1=mx[:])
                nc.vector.tensor_sub(out=sm1[:], in0=z1, in1=mx[:])
                e0 = res.tile([128, G], fp32)
                e1 = res.tile([128, G], fp32)
                nc.scalar.activation(out=e0[:], in_=sm0[:], func=AF.Exp)
                nc.scalar.activation(out=e1[:], in_=sm1[:], func=AF.Exp)
                se = res.tile([128, G], fp32)
                nc.vector.tensor_add(out=se[:], in0=e0[:], in1=e1[:])
                ls = res.tile([128, G], fp32)
                nc.scalar.activation(out=ls[:], in_=se[:], func=AF.Ln)
                nc.vector.tensor_sub(out=sm0[:], in0=sm0[:], in1=ls[:])
                nc.vector.tensor_sub(out=sm1[:], in0=sm1[:], in1=ls[:])
                lpo = res.tile([128, 2 * G], fp32)
                nc.vector.tensor_copy(out=strided(lpo, 0), in_=sm0[:])
                nc.vector.tensor_copy(out=strided(lpo, 1), in_=sm1[:])
                nc.sync.dma_start(out=out_lp[:], in_=lpo[:])

    nc.compile()
    return nc


# ---------------------------------------------------------------- main entry
def _run(x, edge_index, game_indices,
         W1, b1, g1, be1, m1, v1, W2, b2, g2, be2, m2, v2, Wf, bf,
         trace=False, cfg=None):
    from concourse import bass_utils

    if cfg is None:
        cfg = dict(N=N, NPAD=NPAD, SHARD=SHARD, NC=NC, GROUP_EDGES=GROUP_EDGES,
                   GROUP_TILES=GROUP_TILES, H=H, F_IN=F_IN, NCHUNK=NCHUNK)
    cfg.setdefault("NCHUNK", NCHUNK)

    x = np.asarray(x, dtype=np.float32)
    key = ("prep", x.shape, int(np.asarray(edge_index)[0, 0]),
           int(np.asarray(edge_index).sum() % (1 << 31)))
    if key in _CACHE:
        per_core, meta, G, ntiles = _CACHE[key]
    else:
        per_core, meta, G, ntiles = _prepare(x, np.asarray(edge_index), cfg)
        _CACHE.clear()
        _CACHE[key] = (per_core, meta, G, ntiles)

    bkey = ("bass", G, ntiles)
    if bkey in _CACHE:
        nc_a, nc_b = _CACHE[bkey]
    else:
        nc_a = _build(cfg, G, ntiles, "a", meta)
        nc_b = _build(cfg, G, ntiles, "b", meta)
        _CACHE[bkey] = (nc_a, nc_b)

    s1, t1 = _fold_bn(np.asarray(g1), np.asarray(be1), np.asarray(m1),
                      np.asarray(v1), np.asarray(b1))
    s2, t2 = _fold_bn(np.asarray(g2), np.asarray(be2), np.asarray(m2),
                      np.asarray(v2), np.asarray(b2))
    iota = np.broadcast_to(np.arange(128, dtype=np.float16),
                           (128, 128)).copy()
    bf_rep = np.broadcast_to(np.asarray(bf, dtype=np.float32), (128, 2)).copy()

    ncores = cfg["NC"]
    in_maps_a = []
    for c in range(ncores):
        pc = per_core[c]
        in_maps_a.append(dict(
            xT=pc["xT"], W1=np.asarray(W1, np.float32),
            W2=np.asarray(W2, np.float32), s1=s1, t1=t1, iota=iota,
            src1=pc["src1"], dloc=pc["dloc"], dinv_dst=pc["dinv_dst"],
            dinv_nat=pc["dinv_nat"], dinv_padlay=pc["dinv_padlay"],
        ))
    res_a = bass_utils.run_bass_kernel_spmd(
        nc_a, in_maps_a, core_ids=list(range(ncores)), trace=trace)

    in_maps_b = []
    for c in range(ncores):
        pc = per_core[c]
        in_maps_b.append(dict(
            xs2_full_in=res_a.results[c]["xs2_full_out"],
            Wf=np.asarray(Wf, np.float32), bf_rep=bf_rep, s2=s2, t2=t2,
            iota=iota, src2=pc["src2"], dloc=pc["dloc"],
            dinv_dst=pc["dinv_dst"],
        ))
    res_b = bass_utils.run_bass_kernel_spmd(
        nc_b, in_maps_b, core_ids=list(range(ncores)), trace=trace)

    class _Res:
        pass

    res = _Res()
    res.results = res_b.results
    res.exec_time_ns = ((res_a.exec_time_ns or 0) + (res_b.exec_time_ns or 0)) \
        if (res_a.exec_time_ns or res_b.exec_time_ns) else None
    res.parts = (res_a, res_b)

    gi = np.asarray(game_indices, dtype=np.int64)
    cji = meta["pad_cji"][gi]
    lp = np.stack([res_b.results[c]["logp"] for c in range(ncores)])
    out = np.empty((gi.shape[0], 2), dtype=np.float32)
    out[:, 0] = lp[cji[:, 0], cji[:, 2], 2 * cji[:, 1]]
    out[:, 1] = lp[cji[:, 0], cji[:, 2], 2 * cji[:, 1] + 1]
    return out, res


def kernel(**inputs):
    out, _ = _run(**inputs)
    return out


def kernel_profiled(**inputs):
    out, res = _run(**inputs, trace=True)
    return out, res



# revision 24
# speedup vs baseline: 1.0729x; 1.0729x over previous
"""Trainium2 Bass kernel for a 2-layer GCN (EnhancedHockeyGNN) — v2.

Strategy (8 NeuronCores, SPMD, ONE NEFF launch):
  - Stage 1 (replicated, no collective): every core computes the full
    fp16 message table xs1 = (x @ W1) * dinv for all NPAD nodes into 4
    local DRAM quarter-tables (int16 gather indexing needs <=32k rows
    per table).
  - Edge aggregation: edges (incl. self-loops) are sharded by dst owner
    and bin-packed into G groups of <=128 dsts / <=2048 edges. Groups
    are processed in waves of 8; per (wave, quarter) ONE batched
    dma_gather fetches all source rows (descriptor generation is the
    baseline's bottleneck: 994ns + 0.34ns/row vs ~600ns per 128-row
    indirect DMA).  A one-hot matrix (value dinv[dst]) built wide on
    the vector engine turns the segment-sum into PSUM-accumulated fp16
    matmuls producing feature-major aggregates; BN+ReLU folds into one
    scalar-engine activation per group.
  - Layer-2 message table xs2 is staged per group right after its
    layer-1 epilogue and AllGather'd in 4 chunks (Shared outputs),
    overlapping the remaining layer-1 work; layer-2 gathers per
    quarter wait only on their own chunk.
  - Readout computes log-softmax for every node; the host selects the
    requested game_indices rows (pure index routing).
"""
import math

import numpy as np

# ---------------------------------------------------------------- constants
N = 100000
F_IN = 128
H = 128
NC = 8
SHARD = 12544            # multiple of 128; 8 * 12544 = 100352 >= N
NPAD = NC * SHARD
NAT_TILES = NPAD // 128  # 784
NQ = 4                   # index sub-tables / AllGather chunks
Q1_TILES = NAT_TILES // NQ   # 196
Q1_ROWS = Q1_TILES * 128     # 25088 (< 32768 so int16 indices reach)
GROUP_EDGES = 2048
GROUP_DSTS = 128
WAVE = 8                 # groups per wave (PSUM tiles live per wave)
STAGE_CHUNK = 14         # nat tiles per stage-1 DMA (196 % 14 == 0)
EPS = 1e-5

_CACHE = {}


def _chunks(n, k):
    k = min(k, n)
    base, rem = n // k, n % k
    out, lo = [], 0
    for i in range(k):
        hi = lo + base + (1 if i < rem else 0)
        out.append((lo, hi))
        lo = hi
    return out


# ---------------------------------------------------------------- host prep
def _bin_pack(counts, G):
    order = np.argsort(-counts, kind="stable")
    bin_edges = np.zeros(G, dtype=np.int64)
    bin_nodes = np.zeros(G, dtype=np.int64)
    group_of = np.full(counts.shape[0], -1, dtype=np.int32)
    pos_in_group = np.full(counts.shape[0], -1, dtype=np.int32)
    for d in order:
        c = counts[d]
        placed = False
        for b in range(G):
            if bin_edges[b] + c <= GROUP_EDGES and bin_nodes[b] < GROUP_DSTS:
                group_of[d] = b
                pos_in_group[d] = bin_nodes[b]
                bin_edges[b] += c
                bin_nodes[b] += 1
                placed = True
                break
        if not placed:
            return None
    return group_of, pos_in_group


def _prepare(edge_index):
    src = np.asarray(edge_index[0], dtype=np.int64)
    dst = np.asarray(edge_index[1], dtype=np.int64)
    deg = np.bincount(dst, minlength=N).astype(np.float64) + 1.0
    dinv = (1.0 / np.sqrt(deg)).astype(np.float32)
    dinv_pad = np.ones(NPAD, dtype=np.float32)
    dinv_pad[:N] = dinv

    sall = np.concatenate([src, np.arange(N, dtype=np.int64)])
    dall = np.concatenate([dst, np.arange(N, dtype=np.int64)])
    owner = dall // SHARD

    # ----- bin packing per core, shared global G
    Es = [int((owner == c).sum()) for c in range(NC)]
    G = max(int(math.ceil(e / GROUP_EDGES)) for e in Es)
    while True:
        packs = []
        ok = True
        for c in range(NC):
            m = owner == c
            d0 = (dall[m] - c * SHARD).astype(np.int64)
            counts = np.bincount(d0, minlength=SHARD)
            r = _bin_pack(counts, G)
            if r is None:
                ok = False
                break
            packs.append((r[0].astype(np.int64), r[1].astype(np.int64),
                          d0, sall[m]))
        if ok:
            break
        G += 1

    NW = (G + WAVE - 1) // WAVE
    ch_d = _chunks(G, NQ)
    Gq = [hi - lo for lo, hi in ch_d]
    chunk_of_g = np.zeros(G, dtype=np.int64)
    lo_of_chunk = np.array([lo for lo, _ in ch_d], dtype=np.int64)
    for q, (lo, hi) in enumerate(ch_d):
        chunk_of_g[lo:hi] = q

    # ----- per-node L2 location (owner-core group layout, chunked)
    node_g2 = np.zeros(NPAD, dtype=np.int64)
    node_pos2 = np.zeros(NPAD, dtype=np.int64)
    for c in range(NC):
        node_g2[c * SHARD:(c + 1) * SHARD] = packs[c][0]
        node_pos2[c * SHARD:(c + 1) * SHARD] = packs[c][1]
    node_o = np.arange(NPAD, dtype=np.int64) // SHARD
    node_q2 = chunk_of_g[node_g2]
    gq_arr = np.array(Gq, dtype=np.int64)
    node_idx2 = (node_o * gq_arr[node_q2] * 128
                 + (node_g2 - lo_of_chunk[node_q2]) * 128 + node_pos2)

    wave_of_g = np.arange(G, dtype=np.int64) // WAVE

    # ----- per-core edge arrays (both layers share g/dloc/dd; q/idx differ)
    edges = []
    for c in range(NC):
        group_of, pos, d0, s_nodes = packs[c]
        e_g = group_of[d0]
        e_dloc = pos[d0].astype(np.float16)
        e_dd = dinv_pad[c * SHARD + d0].astype(np.float16)
        e_q1 = s_nodes // Q1_ROWS
        e_i1 = (s_nodes % Q1_ROWS).astype(np.int16)
        e_q2 = node_q2[s_nodes]
        e_i2 = node_idx2[s_nodes].astype(np.int16)
        edges.append((e_g, e_dloc, e_dd, e_q1, e_i1, e_q2, e_i2))

    # ----- per-layer slot structure (shared across cores — SPMD)
    KSZ = NW * NQ * G

    def seg_key(e_g, e_q):
        return (wave_of_g[e_g] * NQ + e_q) * G + e_g

    def build_layer(qsel, isel):
        cnts = np.zeros((NC, KSZ), dtype=np.int64)
        for c in range(NC):
            e = edges[c]
            cnts[c] = np.bincount(seg_key(e[0], e[qsel]), minlength=KSZ)
        tseg = -(-cnts.max(axis=0) // 128)  # ceil
        # enumerate calls in (wave, quarter) order
        seg_off = np.zeros(KSZ, dtype=np.int64)  # slot offset per seg key
        calls = []                               # per wave: [(q, k0, segs)]
        k = 0
        for w in range(NW):
            wcalls = []
            g_lo, g_hi = w * WAVE, min((w + 1) * WAVE, G)
            for q in range(NQ):
                k0 = k
                segs = []
                for g in range(g_lo, g_hi):
                    key = (w * NQ + q) * G + g
                    t = int(tseg[key])
                    if t == 0:
                        continue
                    seg_off[key] = k * 128
                    segs.append((g, t))
                    k += t
                if segs:
                    wcalls.append((q, k0, segs))
            calls.append(wcalls)
        ntiles = k
        # per-core tables
        per_core = []
        for c in range(NC):
            e = edges[c]
            key = seg_key(e[0], e[qsel])
            order = np.argsort(key, kind="stable")
            ks = key[order]
            first = np.searchsorted(ks, ks, side="left")
            dest = seg_off[ks] + (np.arange(len(ks)) - first)
            slots_i = np.zeros(ntiles * 128, dtype=np.int16)
            slots_dloc = np.full(ntiles * 128, 300.0, dtype=np.float16)
            slots_dd = np.zeros(ntiles * 128, dtype=np.float16)
            slots_i[dest] = e[isel][order]
            slots_dloc[dest] = e[1][order]
            slots_dd[dest] = e[2][order]
            dloc_tab = slots_dloc.reshape(ntiles, 128).T.copy()
            dd_tab = slots_dd.reshape(ntiles, 128).T.copy()
            idx_tab = np.zeros((128, ntiles * 8), dtype=np.int16)
            for wcalls in calls:
                for q, k0, segs in wcalls:
                    tcall = sum(t for _, t in segs)
                    arr = slots_i[k0 * 128:(k0 + tcall) * 128]
                    idx_tab[0:16, k0 * 8:(k0 + tcall) * 8] = \
                        arr.reshape(-1, 16).T
            for r in range(1, 8):
                idx_tab[16 * r:16 * (r + 1)] = idx_tab[0:16]
            per_core.append((idx_tab, dloc_tab, dd_tab))
        return calls, ntiles, per_core

    calls1, nt1, pc1 = build_layer(3, 4)
    calls2, nt2, pc2 = build_layer(5, 6)

    tmax = 1
    for calls in (calls1, calls2):
        for wcalls in calls:
            for q, k0, segs in wcalls:
                tmax = max(tmax, sum(t for _, t in segs))

    # ----- misc per-core tables
    per_core = []
    for c in range(NC):
        group_of, pos, _, _ = packs[c]
        inv_nodes = np.full(G * 128, -1, dtype=np.int64)
        inv_nodes[group_of * 128 + pos] = np.arange(SHARD)
        valid = inv_nodes >= 0
        vals = np.zeros(G * 128, dtype=np.float32)
        vals[valid] = dinv_pad[c * SHARD + inv_nodes[valid]]
        dinv_padlay = vals.reshape(G, 128).T.copy()
        per_core.append(dict(
            idx1=pc1[c][0], dloc1=pc1[c][1], dd1=pc1[c][2],
            idx2=pc2[c][0], dloc2=pc2[c][1], dd2=pc2[c][2],
            dinv_padlay=dinv_padlay))

    dinv_nat = dinv_pad.reshape(NAT_TILES, 128).T.copy()

    pad_cji = np.zeros((NPAD, 3), dtype=np.int64)
    pad_cji[:, 0] = node_o
    pad_cji[:, 1] = node_g2
    pad_cji[:, 2] = node_pos2

    structure = dict(G=G, NW=NW, ch_d=ch_d, Gq=Gq,
                     calls1=calls1, calls2=calls2,
                     nt1=nt1, nt2=nt2, tmax=tmax)
    return per_core, dinv_nat, structure, pad_cji


def _fold_bn(gamma, beta, mean, var, b):
    s = (gamma / np.sqrt(var + EPS)).astype(np.float32)
    t = ((b - mean) * s + beta).astype(np.float32)
    return s.reshape(H, 1), t.reshape(H, 1)


# ---------------------------------------------------------------- bass build
def _build(st_):
    import os
    dbg = int(os.environ.get("K_DEBUG_STAGE", "5"))
    dbg_edge = os.environ.get("K_DEBUG_EDGE", "full")
    import concourse.bacc as bacc
    import concourse.bass as bass
    import concourse.mybir as mybir
    import concourse.tile as tile

    fp32 = mybir.dt.float32
    fp16 = mybir.dt.float16
    i16 = mybir.dt.int16
    AF = mybir.ActivationFunctionType
    AL = mybir.AluOpType

    G = st_["G"]
    ch_d = st_["ch_d"]
    Gq = st_["Gq"]
    calls1, calls2 = st_["calls1"], st_["calls2"]
    nt1, nt2, TMAX = st_["nt1"], st_["nt2"], st_["tmax"]
    chunk_end = {hi - 1: q for q, (lo, hi) in enumerate(ch_d)}

    nc = bacc.Bacc(None, target_bir_lowering=False, debug=False,
                   num_devices=NC)

    xT_in = nc.dram_tensor("xT", [128, NPAD], fp16, kind="ExternalInput")
    w1_in = nc.dram_tensor("W1", [F_IN, H], fp16, kind="ExternalInput")
    w2_in = nc.dram_tensor("W2", [H, H], fp16, kind="ExternalInput")
    wf_in = nc.dram_tensor("Wf", [H, 2], fp16, kind="ExternalInput")
    s1_in = nc.dram_tensor("s1", [H, 1], fp32, kind="ExternalInput")
    t1_in = nc.dram_tensor("t1", [H, 1], fp32, kind="ExternalInput")
    s2_in = nc.dram_tensor("s2", [H, 1], fp32, kind="ExternalInput")
    t2_in = nc.dram_tensor("t2", [H, 1], fp32, kind="ExternalInput")
    bf_in = nc.dram_tensor("bf_rep", [128, 2], fp32, kind="ExternalInput")
    iota_in = nc.dram_tensor("iota", [128, 128], fp16, kind="ExternalInput")
    dn_in = nc.dram_tensor("dinv_nat", [128, NAT_TILES], fp32,
                           kind="ExternalInput")
    dp_in = nc.dram_tensor("dinv_padlay", [128, G], fp32,
                           kind="ExternalInput")
    idx1_in = nc.dram_tensor("idx1", [128, nt1 * 8], i16,
                             kind="ExternalInput")
    dl1_in = nc.dram_tensor("dloc1", [128, nt1], fp16, kind="ExternalInput")
    dd1_in = nc.dram_tensor("dd1", [128, nt1], fp16, kind="ExternalInput")
    idx2_in = nc.dram_tensor("idx2", [128, nt2 * 8], i16,
                             kind="ExternalInput")
    dl2_in = nc.dram_tensor("dloc2", [128, nt2], fp16, kind="ExternalInput")
    dd2_in = nc.dram_tensor("dd2", [128, nt2], fp16, kind="ExternalInput")
    out_lp = nc.dram_tensor("logp", [128, 2 * G], fp32,
                            kind="ExternalOutput")

    with tile.TileContext(nc) as tc:
        with (
            tc.tile_pool(name="res", bufs=1) as res,
            tc.tile_pool(name="stream", bufs=1) as st,
            tc.tile_pool(name="ps", bufs=1, space="PSUM") as ps,
            tc.tile_pool(name="dram", bufs=1, space="DRAM") as dram,
        ):
            w1_t = res.tile([F_IN, H], fp16)
            w2_t = res.tile([H, H], fp16)
            wf_t = res.tile([H, 2], fp16)
            s1_t = res.tile([H, 1], fp32)
            t1_t = res.tile([H, 1], fp32)
            s2_t = res.tile([H, 1], fp32)
            t2_t = res.tile([H, 1], fp32)
            bf_t = res.tile([128, 2], fp32)
            iota_t = res.tile([128, 128], fp16)
            dn_t = res.tile([128, NAT_TILES], fp32)
            dp_t = res.tile([128, G], fp32)
            dl1_t = res.tile([128, nt1], fp16)
            dd1_t = res.tile([128, nt1], fp16)
            dl2_t = res.tile([128, nt2], fp16)
            dd2_t = res.tile([128, nt2], fp16)
            for t_, i_ in ((w1_t, w1_in), (w2_t, w2_in), (wf_t, wf_in),
                           (s1_t, s1_in), (t1_t, t1_in), (s2_t, s2_in),
                           (t2_t, t2_in), (bf_t, bf_in), (iota_t, iota_in),
                           (dn_t, dn_in), (dp_t, dp_in), (dl1_t, dl1_in),
                           (dd1_t, dd1_in), (dl2_t, dl2_in),
                           (dd2_t, dd2_in)):
                nc.sync.dma_start(out=t_[:], in_=i_[:])

            xs1_q = [dram.tile([Q1_ROWS, H], fp16, name=f"xs1_q{q}")
                     for q in range(NQ)]
            xs2_shard = [dram.tile([Gq[q] * 128, H], fp16,
                                   name=f"xs2_shard{q}") for q in range(NQ)]
            xs2_full = [dram.tile([NC * Gq[q] * 128, H], fp16,
                                  name=f"xs2_full{q}") for q in range(NQ)]

            xs2b = res.tile([128, G * 128], fp16)
            lg = res.tile([128, 2 * G], fp32)
            nc.vector.memset(lg[:], 0.0)

            # ---------------- stage 1: xs1 = (x @ W1) * dinv, all nodes
            for j0 in range(0, NAT_TILES, STAGE_CHUNK):
                xtb = st.tile([128, STAGE_CHUNK * 128], fp16, name="xtb",
                              tag="xtb", bufs=3)
                nc.sync.dma_start(
                    out=xtb[:], in_=xT_in[:, j0 * 128:(j0 + STAGE_CHUNK) * 128])
                xsb = st.tile([128, STAGE_CHUNK, 128], fp16, name="xsb",
                              tag="xsb", bufs=3)
                for t in range(STAGE_CHUNK):
                    j = j0 + t
                    pxs = ps.tile([128, 512], fp32, name="pxs", tag="pg",
                                  bufs=8)
                    nc.tensor.matmul(pxs[:, :H],
                                     xtb[:, t * 128:(t + 1) * 128],
                                     w1_t[:], start=True, stop=True)
                    nc.vector.tensor_scalar(
                        out=xsb[:, t, :], in0=pxs[:, :H],
                        scalar1=dn_t[:, j:j + 1], scalar2=None,
                        op0=AL.mult)
                q = j0 // Q1_TILES
                r0 = (j0 - q * Q1_TILES) * 128
                dest = bass.AP(xs1_q[q][:].tensor, r0 * H,
                               [[H, 128], [128 * H, STAGE_CHUNK], [1, H]])
                nc.sync.dma_start(out=dest, in_=xsb[:])

            # ---------------- generic edge layer
            def iota_bc(tcall):
                a = iota_t[:]
                return bass.AP(a.tensor, a.offset,
                               [a.ap[0], [0, tcall], [1, 128]])

            def edge_layer(calls, xs_tiles, dl_t, dd_t, idx_in, s_t, t_t,
                           htag, post_wave):
                for w, wcalls in enumerate(calls):
                    remaining = {}
                    for q, k0, segs in wcalls:
                        for g, tg in segs:
                            remaining[g] = remaining.get(g, 0) + tg
                    glist = sorted(remaining)
                    pgs = {g: ps.tile([128, 512], fp32, name="pgb",
                                      tag="pg", bufs=8) for g in glist}
                    started = set()

                    def pg_ap(g):
                        return pgs[g][:, :H]

                    for q, k0, segs in wcalls:
                        tcall = sum(tg for _, tg in segs)
                        ni = tcall * 128
                        idxsb = st.tile([128, TMAX * 8], i16, name="idx",
                                        tag="idx", bufs=4)
                        nc.sync.dma_start(
                            out=idxsb[:, :tcall * 8],
                            in_=idx_in[:, k0 * 8:(k0 + tcall) * 8])
                        msg = st.tile([128, TMAX, 128], fp16, name="msg",
                                      tag="msg", bufs=3)
                        src_ap = xs_tiles[q][:]
                        nc.gpsimd.dma_gather(
                            msg[:, :tcall, :], src_ap, idxsb[:, :tcall * 8],
                            ni, ni, H, elem_step=src_ap.ap[0][0],
                            single_packet=False)
                        if dbg_edge == "gather":
                            continue
                        oh = st.tile([128, TMAX, 128], fp16, name="oh",
                                     tag="oh", bufs=3)
                        nc.vector.tensor_tensor(
                            out=oh[:, :tcall, :],
                            in0=dl_t[:, k0:k0 + tcall].to_broadcast(
                                [128, tcall, 128]),
                            in1=iota_bc(tcall),
                            op=AL.is_equal)
                        nc.vector.tensor_tensor(
                            out=oh[:, :tcall, :],
                            in0=oh[:, :tcall, :],
                            in1=dd_t[:, k0:k0 + tcall].to_broadcast(
                                [128, tcall, 128]),
                            op=AL.mult)
                        if dbg_edge == "onehot":
                            continue
                        tl = 0
                        for g, tg in segs:
                            for _ in range(tg):
                                first = g not in started
                                started.add(g)
                                nc.tensor.matmul(
                                    pg_ap(g), msg[:, tl, :], oh[:, tl, :],
                                    start=first, stop=(remaining[g] == 1))
                                remaining[g] -= 1
                                tl += 1
                    if dbg_edge in ("gather", "onehot", "mm"):
                        continue
                    hTw = st.tile([128, WAVE * 128], fp16, name=htag,
                                  tag=htag, bufs=2)
                    for i, g in enumerate(glist):
                        nc.scalar.activation(
                            out=hTw[:, i * 128:(i + 1) * 128],
                            in_=pg_ap(g), func=AF.Relu,
                            bias=t_t[:], scale=s_t[:])
                    post_wave(glist, hTw)

            # ---------------- layer 1 (+ xs2 staging and chunked AllGather)
            def post1(glist, hTw):
                for i, g in enumerate(glist):
                    pxs2 = ps.tile([128, 512], fp32, name="pxs2", tag="pg",
                                   bufs=8)
                    nc.tensor.matmul(pxs2[:, :H],
                                     hTw[:, i * 128:(i + 1) * 128],
                                     w2_t[:], start=True, stop=True)
                    nc.vector.tensor_scalar(
                        out=xs2b[:, g * 128:(g + 1) * 128],
                        in0=pxs2[:, :H],
                        scalar1=dp_t[:, g:g + 1], scalar2=None, op0=AL.mult)
                    if g in chunk_end and dbg >= 3:
                        q = chunk_end[g]
                        lo, hi = ch_d[q]
                        dest = bass.AP(xs2_shard[q][:].tensor, 0,
                                       [[H, 128], [128 * H, hi - lo], [1, H]])
                        nc.sync.dma_start(
                            out=dest,
                            in_=xs2b[:].rearrange("p (j f) -> p j f",
                                                  f=H)[:, lo:hi, :])
                        if dbg >= 4:
                            nc.gpsimd.collective_compute(
                                "AllGather", mybir.AluOpType.bypass,
                                replica_groups=[list(range(NC))],
                                ins=[xs2_shard[q][:].opt()],
                                outs=[xs2_full[q][:].opt()],
                            )

            if dbg >= 2:
                edge_layer(calls1, xs1_q, dl1_t, dd1_t, idx1_in, s1_t, t1_t,
                           "hT1", post1)

            # ---------------- layer 2 (+ readout)
            def post2(glist, hTw):
                for i, g in enumerate(glist):
                    plg = ps.tile([128, 512], fp32, name="plg", tag="pg",
                                  bufs=8)
                    nc.tensor.matmul(plg[:, 0:2],
                                     hTw[:, i * 128:(i + 1) * 128],
                                     wf_t[:], start=True, stop=True)
                    nc.vector.tensor_add(out=lg[:, 2 * g:2 * g + 2],
                                         in0=plg[:, 0:2],
                                         in1=bf_t[:])

            if dbg >= 5:
                edge_layer(calls2, xs2_full, dl2_t, dd2_t, idx2_in, s2_t,
                           t2_t, "hT2", post2)

            # ---------------- log-softmax over the 2 logits per node
            def strided(base, start):
                a = base[:]
                return bass.AP(a.tensor, a.offset + start, [a.ap[0], [2, G]])

            z0, z1 = strided(lg, 0), strided(lg, 1)
            mx = res.tile([128, G], fp32)
            nc.vector.tensor_tensor(out=mx[:], in0=z0, in1=z1, op=AL.max)
            sm0 = res.tile([128, G], fp32)
            sm1 = res.tile([128, G], fp32)
            nc.vector.tensor_sub(out=sm0[:], in0=z0, in1=mx[:])
            nc.vector.tensor_sub(out=sm1[:], in0=z1, in1=mx[:])
            e0 = res.tile([128, G], fp32)
            e1 = res.tile([128, G], fp32)
            nc.scalar.activation(out=e0[:], in_=sm0[:], func=AF.Exp)
            nc.scalar.activation(out=e1[:], in_=sm1[:], func=AF.Exp)
            se = res.tile([128, G], fp32)
            nc.vector.tensor_add(out=se[:], in0=e0[:], in1=e1[:])
            ls = res.tile([128, G], fp32)
            nc.scalar.activation(out=ls[:], in_=se[:], func=AF.Ln)
            nc.vector.tensor_sub(out=sm0[:], in0=sm0[:], in1=ls[:])
            nc.vector.tensor_sub(out=sm1[:], in0=sm1[:], in1=ls[:])
            lpo = res.tile([128, 2 * G], fp32)
            nc.vector.tensor_copy(out=strided(lpo, 0), in_=sm0[:])
            nc.vector.tensor_copy(out=strided(lpo, 1), in_=sm1[:])
            nc.sync.dma_start(out=out_lp[:], in_=lpo[:])

    nc.compile()
    return nc


# ---------------------------------------------------------------- main entry
def _run(x, edge_index, game_indices,
         W1, b1, g1, be1, m1, v1, W2, b2, g2, be2, m2, v2, Wf, bf,
         trace=False):
    from concourse import bass_utils

    ei = np.asarray(edge_index)
    key = ("prep", int(ei[0, 0]), int(ei.sum() % (1 << 31)))
    if key in _CACHE:
        per_core, dinv_nat, structure, pad_cji = _CACHE[key]
    else:
        per_core, dinv_nat, structure, pad_cji = _prepare(ei)
        _CACHE.clear()
        _CACHE[key] = (per_core, dinv_nat, structure, pad_cji)

    skey = ("bass", structure["G"], structure["nt1"], structure["nt2"],
            structure["tmax"])
    if skey in _CACHE:
        nc = _CACHE[skey]
    else:
        nc = _build(structure)
        _CACHE[skey] = nc

    G = structure["G"]

    x = np.asarray(x, dtype=np.float32)
    xT = np.zeros((128, NPAD), dtype=np.float16)
    xT[:, :N] = x.T
    s1, t1 = _fold_bn(np.asarray(g1), np.asarray(be1), np.asarray(m1),
                      np.asarray(v1), np.asarray(b1))
    s2, t2 = _fold_bn(np.asarray(g2), np.asarray(be2), np.asarray(m2),
                      np.asarray(v2), np.asarray(b2))
    iota = np.broadcast_to(np.arange(128, dtype=np.float16),
                           (128, 128)).copy()
    bf_rep = np.broadcast_to(np.asarray(bf, dtype=np.float32), (128, 2)).copy()
    w1h = np.asarray(W1, np.float16)
    w2h = np.asarray(W2, np.float16)
    wfh = np.asarray(Wf, np.float16)

    in_maps = []
    for c in range(NC):
        pc = per_core[c]
        in_maps.append(dict(
            xT=xT, W1=w1h, W2=w2h, Wf=wfh, s1=s1, t1=t1, s2=s2, t2=t2,
            bf_rep=bf_rep, iota=iota, dinv_nat=dinv_nat,
            dinv_padlay=pc["dinv_padlay"],
            idx1=pc["idx1"], dloc1=pc["dloc1"], dd1=pc["dd1"],
            idx2=pc["idx2"], dloc2=pc["dloc2"], dd2=pc["dd2"],
        ))
    res = bass_utils.run_bass_kernel_spmd(
        nc, in_maps, core_ids=list(range(NC)), trace=trace)

    gi = np.asarray(game_indices, dtype=np.int64)
    cji = pad_cji[gi]
    lp = np.stack([res.results[c]["logp"] for c in range(NC)])
    out = np.empty((gi.shape[0], 2), dtype=np.float32)
    out[:, 0] = lp[cji[:, 0], cji[:, 2], 2 * cji[:, 1]]
    out[:, 1] = lp[cji[:, 0], cji[:, 2], 2 * cji[:, 1] + 1]
    return out, res


def kernel(**inputs):
    out, _ = _run(**inputs)
    return out


def kernel_profiled(**inputs):
    out, res = _run(**inputs, trace=True)
    return out, res
